# revision 1
# baseline (speedup 1.0000x reference)
"""Causal multi-head attention block (GPT-style) for Trainium2, 8 NeuronCores.

Problem: x[4,2048,768] -> qkv = x@W_attn+b_attn -> 12-head causal attention
         -> y@W_proj+b_proj -> out[4,2048,768]   (fp32 I/O)

Sharding: 4 batches x 2 head-groups (6 heads each). c_attn column-sharded,
c_proj row-sharded over head groups; AllReduce(add) over core pairs after
c_proj. Core c = 2*b + g handles batch b, heads 6g..6g+5.

Per-core kernel (all matmuls bf16, fp32 accumulation):
  1. QKV^T = Wa_g^T @ x^T: Q^T,K^T [384,2048] col-tile-major; V [2048,384]
     stored head-strided with a ones-column per head ([128,390] tiles).
  2. Flash-style causal attention in transposed-score orientation:
     S^T[k,q] blocks via row-packed pair matmuls (K=64 head A rows 0-63 /
     head B rows 64-127), exp on ScalarE (PSUM->SBUF bf16, scale=1/8),
     diagonal-band masking via precomputed shifted-tril mask multiply,
     y_u^T accumulation via V_aug[128,65] matmuls (65th row = softmax
     normalizer n[q] thanks to the ones column).
  3. Normalize: r = 1/n (DVE), partition-broadcast r via DMA, y^T = y_u^T*r.
  4. proj: out_partial[s,768] = sum_pairs yT_pair^T @ Wp_pair, + b_proj.
  5. AllReduce(add) over {2b, 2b+1} in 512-row chunks, overlapped.

The walrus build here allows only one sync-wait per instruction; a post-pass
(legalize_waits) hoists extra waits onto single-wait NOPs.
"""
import numpy as np
import ml_dtypes

import concourse.bass as bass
import concourse.tile as tile
from concourse import mybir
from concourse.bass_utils import run_bass_kernel_spmd
from concourse import mybir as mb

BF16 = mybir.dt.bfloat16
F32 = mybir.dt.float32

B, S, D = 4, 2048, 768
H, HD = 12, 64
G = 2                 # head groups
HL = H // G           # heads per core = 6
DL = HL * HD          # local head dims = 384
NP = HL // 2          # head pairs per core = 3
P = 128
QT = 512              # q tile
KT = 128              # k tile
N_CORES = 8
NI = S // QT          # 4 q tiles
NS = S // P           # 16 s tiles
NDT = D // P          # 6 D tiles
USE_AR = True
REPEAT = 1  # emit the whole computation N times (for timing-slope calibration)


def _legalize_waits(nc):
    n_split = 0
    for f in nc.m.functions:
        for bb in f.blocks:
            insts = list(bb.instructions)
            out = []
            changed = False
            for inst in insts:
                si = inst.sync_info
                if si is not None:
                    waits = list(si.on_wait)
                    if len(waits) > 1:
                        for w in waits[:-1]:
                            nop = mb.InstNoOp(name=f"I-wsplit-{nc.next_id()}", ins=[], outs=[])
                            nop.engine = inst.engine
                            nop.sync_info = mb.SyncInfo(on_wait=[w], on_update=[])
                            out.append(nop)
                            n_split += 1
                        inst.sync_info = mb.SyncInfo(on_wait=[waits[-1]], on_update=list(si.on_update))
                        changed = True
                out.append(inst)
            if changed:
                bb.instructions = out
    return n_split


def _build():
    nc = bass.Bass("TRN2", target_bir_lowering=False, debug=False, num_devices=N_CORES)

    xT = nc.dram_tensor("xT", [D, S], BF16, kind="ExternalInput").ap()
    wa = nc.dram_tensor("wa", [D, 3 * DL], BF16, kind="ExternalInput").ap()
    ba = nc.dram_tensor("ba", [P, 9], F32, kind="ExternalInput").ap()
    bv = nc.dram_tensor("bv", [P, DL], F32, kind="ExternalInput").ap()
    wp = nc.dram_tensor("wp", [DL, D], BF16, kind="ExternalInput").ap()
    bp = nc.dram_tensor("bp", [P, D], F32, kind="ExternalInput").ap()
    msk = nc.dram_tensor("msk", [P, 896], BF16, kind="ExternalInput").ap()
    out = nc.dram_tensor("out", [S, D], F32, kind="ExternalOutput").ap()
    if USE_AR:
        ar_in = nc.dram_tensor("ar_in", [S, D], F32).ap()
        # one output tensor per AllReduce chunk: Tile's DRAM dependency
        # tracking is tensor-granular, so a shared ar_out would make every
        # final copy wait for the LAST AllReduce
        ar_outs = [nc.dram_tensor(f"ar_out{c}", [QT, D], F32).ap() for c in range(NI)]

    with tile.TileContext(nc) as tc:
        with (
            tc.tile_pool(name="wgt", bufs=1) as wpool,
            tc.tile_pool(name="qkv", bufs=1) as qkvpool,
            tc.tile_pool(name="pt", bufs=4) as ptpool,
            tc.tile_pool(name="yt", bufs=2) as ytpool,
            tc.tile_pool(name="nrm", bufs=3) as nrmpool,
            tc.tile_pool(name="ob", bufs=3) as obpool,
            tc.tile_pool(name="scp", bufs=3, space="PSUM") as scpool,
            tc.tile_pool(name="avp", bufs=1, space="PSUM") as avpool,
        ):
            # ---- phase 0: load weights/constants ----
            wak = []
            for t in range(NDT):
                wt_sb = wpool.tile([P, 3 * DL], BF16, tag=f"wak{t}")
                nc.sync.dma_start(wt_sb[:], wa[bass.ts(t, P), :])
                wak.append(wt_sb)
            xk = []
            for t in range(NDT):
                xt_sb = wpool.tile([P, S], BF16, tag=f"xk{t}")
                nc.sync.dma_start(xt_sb[:, 0 : S // 2], xT[bass.ts(t, P), 0 : S // 2])
                xk.append(xt_sb)
            for t in range(NDT):
                nc.sync.dma_start(xk[t][:, S // 2 :], xT[bass.ts(t, P), S // 2 :])
            wpp = []
            for p in range(NP):
                wp_sb = wpool.tile([P, D], BF16, tag=f"wpp{p}")
                nc.sync.dma_start(wp_sb[:], wp[bass.ts(p, P), :])
                wpp.append(wp_sb)
            ba_sb = wpool.tile([P, 9], F32, tag="ba")
            nc.sync.dma_start(ba_sb[:], ba[:])
            bv_sb = wpool.tile([P, DL], F32, tag="bv")
            nc.sync.dma_start(bv_sb[:], bv[:])
            bp_sb = wpool.tile([P, D], F32, tag="bp")
            nc.sync.dma_start(bp_sb[:], bp[:])
            msk_sb = wpool.tile([P, 896], BF16, tag="msk")
            nc.sync.dma_start(msk_sb[:], msk[:])
            ones_sb = wpool.tile([1, HD], mybir.dt.float32r, tag="ones")
            nc.vector.memset(ones_sb[:].bitcast(F32), 1.0)
            # prewarm ScalarE's exp table set during the QKV phase so the
            # first attention exp doesn't pay the ~2.7us ACT_TABLE_LOAD
            warm_sb = wpool.tile([1, 2], F32, tag="warm")
            nc.vector.memset(warm_sb[:], 0.0)
            nc.scalar.activation(warm_sb[:, 1:2], warm_sb[:, 0:1],
                                 mybir.ActivationFunctionType.Exp)

            # ---- phase 1: Q^T, K^T  (col-tile m: 0-2 = Q pairs, 3-5 = K pairs)
            qt_t, kt_t = [None] * NP, [None] * NP

            def emit_qk(m, n2):
                if n2 == 0:
                    dst = qkvpool.tile([P, S], BF16, tag=f"qkvT{m}")
                    if m < NP:
                        qt_t[m] = dst
                    else:
                        kt_t[m - NP] = dst
                else:
                    dst = qt_t[m] if m < NP else kt_t[m - NP]
                ps = scpool.tile([P, 1024], F32, tag="sc")
                for half in range(2):
                    n = 2 * n2 + half
                    for t in range(NDT):
                        nc.tensor.matmul(
                            ps[:, bass.ts(half, QT)],
                            lhsT=wak[t][:, bass.ts(m, P)],
                            rhs=xk[t][:, bass.ts(n, QT)],
                            start=(t == 0),
                            stop=(t == NDT - 1),
                        )
                for half in range(2):
                    n = 2 * n2 + half
                    nc.vector.tensor_scalar_add(
                        dst[:, bass.ts(n, QT)],
                        ps[:, bass.ts(half, QT)],
                        ba_sb[:, m : m + 1],
                    )

            # V s-tiles, emitted lazily before the attention i-tile that needs them
            v_t = [None] * NS

            def emit_v(s):
                ps = scpool.tile([P, 1024], F32, tag="sc")
                for t in range(NDT):
                    nc.tensor.matmul(
                        ps[:, 0:DL],
                        lhsT=xk[t][:, bass.ts(s, P)],
                        rhs=wak[t][:, 2 * DL : 3 * DL],
                        start=(t == 0),
                        stop=(t == NDT - 1),
                    )
                vt = qkvpool.tile([P, HL * 65], BF16, tag=f"v{s}")
                vt3 = vt[:].rearrange("p (h x) -> p h x", h=HL)
                nc.vector.tensor_add(
                    vt3[:, :, 0:HD],
                    ps[:, 0:DL].rearrange("p (h x) -> p h x", h=HL),
                    bv_sb[:].rearrange("p (h x) -> p h x", h=HL),
                )
                nc.vector.memset(vt3[:, :, HD : HD + 1], 1.0)
                v_t[s] = vt

            yt_t = [None] * NP

            def emit_attn(i, p):
                njt = 4 * (i + 1)      # k tiles for this q tile
                noff = njt - 4         # full-width off-diagonal j tiles
                avA = avpool.tile([65, QT], F32, tag="avA")
                avB = avpool.tile([65, QT], F32, tag="avB")
                hA, hB = 2 * p, 2 * p + 1
                scs = {}
                # group list: full-width pairs of j, then 2 narrowed diagonal
                # groups (mi=0:N=512, mi=1:384 | mi=2:256, mi=3:128) -- the
                # diagonal band only has valid scores at q_local >= k + 128*mi
                groups = [("f", g) for g in range(noff // 2)] + [("d", 0), ("d", 1)]
                # (block j, N, q_offset_global, offset inside sc/pt tile)
                def blocks_of(kind, g):
                    if kind == "f":
                        return [(2 * g + jj, QT, i * QT, jj * QT) for jj in range(2)]
                    res = []
                    for idx in range(2):
                        mi = 2 * g + idx
                        n = QT - 128 * mi
                        off = 0 if idx == 0 else (QT if g == 0 else 256)
                        res.append((4 * i + mi, n, i * QT + 128 * mi, off))
                    return res

                def emit_scores(kind, g):
                    scA = scpool.tile([P, 1024], F32, tag="sc")
                    scB = scpool.tile([P, 1024], F32, tag="sc")
                    for j, n, q0, off in blocks_of(kind, g):
                        nc.tensor.matmul(
                            scA[:, off : off + n],
                            lhsT=kt_t[p][0:HD, bass.ts(j, P)],
                            rhs=qt_t[p][0:HD, q0 : q0 + n],
                            start=True, stop=True,
                        )
                        nc.tensor.matmul(
                            scB[:, off : off + n],
                            lhsT=kt_t[p][HD:P, bass.ts(j, P)],
                            rhs=qt_t[p][HD:P, q0 : q0 + n],
                            start=True, stop=True,
                        )
                    scs[(kind, g)] = (scA, scB)

                emit_scores(*groups[0])
                for gi, (kind, g) in enumerate(groups):
                    if gi + 1 < len(groups):
                        emit_scores(*groups[gi + 1])
                    scA, scB = scs.pop((kind, g))
                    width = sum(b[1] for b in blocks_of(kind, g))
                    woff = min(b[3] for b in blocks_of(kind, g))
                    ptA = ptpool.tile([P, 1024], BF16, tag="pt")
                    ptB = ptpool.tile([P, 1024], BF16, tag="pt")
                    nc.scalar.activation(ptA[:, woff : woff + width], scA[:, woff : woff + width],
                                         mybir.ActivationFunctionType.Exp, scale=0.125)
                    nc.scalar.activation(ptB[:, woff : woff + width], scB[:, woff : woff + width],
                                         mybir.ActivationFunctionType.Exp, scale=0.125)
                    if kind == "d":
                        for j, n, q0, off in blocks_of(kind, g):
                            for pt in (ptA, ptB):
                                nc.vector.tensor_mul(
                                    pt[:, off : off + n],
                                    pt[:, off : off + n],
                                    msk_sb[:, 384 : 384 + n],
                                )
                    for j, n, q0, off in blocks_of(kind, g):
                        qoff = q0 - i * QT
                        nc.tensor.matmul(
                            avA[:, qoff : qoff + n],
                            lhsT=v_t[j][:, hA * 65 : (hA + 1) * 65],
                            rhs=ptA[:, off : off + n],
                            start=(j == 0), stop=(j == njt - 1),
                        )
                        nc.tensor.matmul(
                            avB[:, qoff : qoff + n],
                            lhsT=v_t[j][:, hB * 65 : (hB + 1) * 65],
                            rhs=ptB[:, off : off + n],
                            start=(j == 0), stop=(j == njt - 1),
                        )

                # normalize: yT_pair[0:64] = avA[0:64]/avA[64], [64:128] = avB/..
                # n-row -> SBUF, broadcast to 64 partitions via ones-matmul,
                # reciprocal PSUM->SBUF, then multiply.
                ytp = ytpool.tile([P, QT], BF16, tag=f"yt{p}")
                n2 = nrmpool.tile([1, 2 * QT], mybir.dt.float32r, tag="n2")
                nc.vector.tensor_copy(n2[:, 0:QT], avA[64:65, :])
                nc.vector.tensor_copy(n2[:, QT:], avB[64:65, :])
                nb = scpool.tile([P, 1024], F32, tag="sc")
                nc.tensor.matmul(nb[0:HD, 0:QT], lhsT=ones_sb[:], rhs=n2[:, 0:QT], start=True, stop=True)
                nc.tensor.matmul(nb[0:HD, QT:], lhsT=ones_sb[:], rhs=n2[:, QT:], start=True, stop=True)
                rb = nrmpool.tile([HD, 2 * QT], F32, tag="rb")
                nc.vector.reciprocal(rb[:], nb[0:HD, :])
                nc.vector.tensor_mul(ytp[0:HD, :], avA[0:HD, :], rb[:, 0:QT])
                tmpB = nrmpool.tile([HD, QT], BF16, tag="tmpB")
                nc.vector.tensor_mul(tmpB[:], avB[0:HD, :], rb[:, QT:])
                nc.sync.dma_start(ytp[HD:P, :], tmpB[:])
                yt_t[p] = ytp

            # AllReduce chunk boundaries (rows): skewed so the last chunk is
            # tiny — its AR + final copy are the serial tail of the kernel.
            ar_chunks = []

            def emit_ar(r0, r1):
                c = r0 // QT
                nc.gpsimd.collective_compute(
                    "AllReduce",
                    mybir.AluOpType.add,
                    replica_groups=[[0, 1], [2, 3], [4, 5], [6, 7]],
                    ins=[ar_in[r0:r1, :].opt()],
                    outs=[ar_outs[c][:].opt()],
                )

            def emit_copy(r):
                # output copy for one 128-row block, bounced through SBUF
                # (model prices DRAM->DRAM far above two SBUF hops)
                c, rr = divmod(r, QT)
                oc = obpool.tile([P, D], F32, tag="oc")
                nc.sync.dma_start(oc[:], ar_outs[c][rr : rr + P, :])
                nc.sync.dma_start(out[r : r + P, :], oc[:])

            def emit_proj(i):
                for ss in range(QT // P):
                    ps = scpool.tile([P, 1024], F32, tag="sc")
                    for p in range(NP):
                        nc.tensor.matmul(
                            ps[:, 0:512],
                            lhsT=yt_t[p][:, bass.ts(ss, P)],
                            rhs=wpp[p][:, 0:512],
                            start=(p == 0), stop=(p == NP - 1),
                        )
                        nc.tensor.matmul(
                            ps[:, 512:768],
                            lhsT=yt_t[p][:, bass.ts(ss, P)],
                            rhs=wpp[p][:, 512:768],
                            start=(p == 0), stop=(p == NP - 1),
                        )
                    ob = obpool.tile([P, D], F32, tag="ob")
                    nc.vector.tensor_add(ob[:], ps[:, 0:D], bp_sb[:])
                    row0 = i * QT + ss * P
                    dst = ar_in if USE_AR else out
                    nc.sync.dma_start(dst[row0 : row0 + P, :], ob[:])
                    if USE_AR:
                        done = row0 + P
                        while ar_chunks and ar_chunks[0][1] <= done:
                            r0, r1 = ar_chunks.pop(0)
                            if r1 == S:
                                # before the last AllReduce is emitted, flush
                                # the earlier chunks' output copies: tile
                                # serializes later-emitted DMAs behind
                                # collectives regardless of data deps
                                for r in range(0, r0, P):
                                    emit_copy(r)
                            emit_ar(r0, r1)
                            if r1 == S:
                                for r in range(r0, S, P):
                                    emit_copy(r)

            # ---- main interleaved schedule ----
            for _rep in range(REPEAT):
                ar_chunks.clear()
                ar_chunks.extend([(0, 512), (512, 1024), (1024, 1536), (1536, 2048)])
                for m in range(2 * NP):
                    emit_qk(m, 0)
                    emit_qk(m, 1)
                for s in range(4):
                    emit_v(s)
                for i in range(NI):
                    for p in range(NP):
                        emit_attn(i, p)
                    if i + 1 < NI:
                        for s in range(4 * (i + 1), 4 * (i + 2)):
                            emit_v(s)
                    emit_proj(i)

    _legalize_waits(nc)
    return nc


_NC_CACHE = {}


def _get_nc():
    if "nc" not in _NC_CACHE:
        _NC_CACHE["nc"] = _build()
    return _NC_CACHE["nc"]


def _prep_inputs(x, W_attn, b_attn, W_proj, b_proj):
    bf = ml_dtypes.bfloat16
    x = np.asarray(x, np.float32)
    W_attn = np.asarray(W_attn, np.float32)
    b_attn = np.asarray(b_attn, np.float32)
    W_proj = np.asarray(W_proj, np.float32)
    b_proj = np.asarray(b_proj, np.float32)

    t_idx = np.arange(896)[None, :]
    k_idx = np.arange(P)[:, None]
    mask = (t_idx >= k_idx + 384).astype(bf)

    in_maps = []
    for c in range(N_CORES):
        b, g = divmod(c, 2)
        cols = slice(DL * g, DL * g + DL)
        xT = np.ascontiguousarray(x[b].T).astype(bf)
        wa = np.concatenate(
            [W_attn[:, 0:D][:, cols], W_attn[:, D : 2 * D][:, cols], W_attn[:, 2 * D :][:, cols]],
            axis=1,
        ).astype(bf)
        ba_sl = np.concatenate(
            [b_attn[0:D][cols], b_attn[D : 2 * D][cols], b_attn[2 * D :][cols]]
        ).astype(np.float32)
        ba2 = np.ascontiguousarray(ba_sl[: 2 * DL].reshape(6, P).T)
        ba9 = np.zeros((P, 9), np.float32)
        ba9[:, :6] = ba2
        bv_b = np.ascontiguousarray(np.broadcast_to(ba_sl[2 * DL :], (P, DL))).astype(np.float32)
        wp_c = np.ascontiguousarray(W_proj[cols, :]).astype(bf)
        bp_full = b_proj if g == 0 else np.zeros_like(b_proj)
        bp_b = np.ascontiguousarray(np.broadcast_to(bp_full, (P, D))).astype(np.float32)
        in_maps.append(
            {
                "xT": xT,
                "wa": wa,
                "ba": ba9,
                "bv": bv_b,
                "wp": wp_c,
                "bp": bp_b,
                "msk": mask,
            }
        )
    return in_maps


def kernel(x, W_attn, b_attn, W_proj, b_proj):
    in_maps = _prep_inputs(x, W_attn, b_attn, W_proj, b_proj)
    nc = _get_nc()
    res = run_bass_kernel_spmd(nc, in_maps, list(range(N_CORES)))
    if USE_AR:
        out = np.stack([res.results[2 * b]["out"] for b in range(B)])
    else:
        out = np.stack(
            [res.results[2 * b]["out"] + res.results[2 * b + 1]["out"] for b in range(B)]
        )
    return out.astype(np.float32)



# revision 10
# speedup vs baseline: 1.2043x; 1.2043x over previous
"""Causal multi-head attention block (GPT-style) for Trainium2, 8 NeuronCores.

Problem: x[4,2048,768] -> qkv = x@W_attn+b_attn -> 12-head causal attention
         -> y@W_proj+b_proj -> out[4,2048,768]   (fp32 I/O)

Sharding: 4 batches x 2 head-groups (6 heads each); core c = 2*b + g handles
batch b, heads 6g..6g+5. c_proj row-sharded; AllReduce(add) over core pairs.

v2 kernel — fp8 DoubleRow (DR) matmuls everywhere the error budget allows:
  1. Q,K projection: fp8e4 DR over d-chunk pairs (contraction 256/instr,
     0.5 cyc/row). W_attn[q,k] scaled by SA=16 on host so fp8 sees ~N(0,0.3).
     PSUM -> (DVE +bias, fp8 out) qraw[128,S] -> DMA relayout to [32,2,S]
     per head so scores can run DR with K=32x2=64 (exact head_dim).
  2. V projection: bf16 (V feeds y almost linearly -> needs > fp8 accuracy),
     then split v = v_hi + v_lo (both fp8e4); the AV matmul consumes
     [v_hi|v_lo] as the two DR k-tiles with the SAME pt tile (stride-0 dim)
     => exact-to-fp8-residual V at DR speed.
  3. Scores S^T[k,q] per 128-k block via one DR instr per head (K=32x2).
     Causal diagonal: 128-wide staircase chunks get a shared tril mask
     ([128,128], 0/-3e5) ADDED in PSUM by DVE before exp (no post-mask).
  4. exp on ACT (the bottleneck engine): wide [128,<=1024] calls, fp8 out.
  5. AV transposed: out y_u[q-part, 65] per (q-128-chunk, head, j):
     lhsT = pt (stationary, stride-0 doubled), rhs = [v_hi|v_lo]. The ones
     column of v_hi makes col 64 the softmax denominator n[q] -- a
     per-partition scalar: normalize = DVE recip[128,4] + one strided mul.
  6. y chunks [128 q, 128 hd] -> PE transpose (bf16) -> y^T for proj.
  7. proj: bf16, contraction over the core's 3 pair-tiles, PSUM->DRAM DMA
     (b_proj+b_v@W_proj folded on host; DVE add only if nonzero).
  8. AllReduce(add) in 2 chunks: rows [0:1024] mid-kernel (hidden),
     [1024:2048] as the tail.

The walrus build allows only one sync-wait per instruction; legalize_waits
hoists extras onto single-wait NOPs.
"""
import numpy as np
import ml_dtypes

import concourse.bass as bass
import concourse.tile as tile
from concourse import mybir
from concourse.bass_utils import run_bass_kernel_spmd
from concourse import mybir as mb

BF16 = mybir.dt.bfloat16
F8 = mybir.dt.float8e4
F32 = mybir.dt.float32
DR = mybir.MatmulPerfMode.DoubleRow

B, S, D = 4, 2048, 768
H, HD = 12, 64
G = 2                 # head groups
HL = H // G           # heads per core = 6
DL = HL * HD          # local head dims = 384
NP = HL // 2          # head pairs per core = 3
P = 128
QT = 512              # q tile
NI = S // QT          # 4 q tiles
NS = S // P           # 16 k/s tiles
NDT = D // P          # 6 d tiles
NDP = NDT // 2        # 3 d pair tiles
N_CORES = 8
SA = 16.0             # host scale on W_attn[q,k] before fp8
ESC = 0.125 / (SA * SA)   # exp scale undoing SA^2 and 1/sqrt(hd)
MASKC = -3.0e5
# diag remainder layout inside scR (bank-crossing-free): mi -> col offset
REM_OFF = {0: 0, 1: 512, 2: 384}
REM_W = {0: 384, 1: 256, 2: 128}


def _legalize_waits(nc):
    n_split = 0
    for f in nc.m.functions:
        for bb in f.blocks:
            insts = list(bb.instructions)
            out = []
            changed = False
            for inst in insts:
                si = inst.sync_info
                if si is not None:
                    waits = list(si.on_wait)
                    if len(waits) > 1:
                        for w in waits[:-1]:
                            nop = mb.InstNoOp(name=f"I-wsplit-{nc.next_id()}", ins=[], outs=[])
                            nop.engine = inst.engine
                            nop.sync_info = mb.SyncInfo(on_wait=[w], on_update=[])
                            out.append(nop)
                            n_split += 1
                        inst.sync_info = mb.SyncInfo(on_wait=[waits[-1]], on_update=list(si.on_update))
                        changed = True
                out.append(inst)
            if changed:
                bb.instructions = out
    return n_split


def _build(qk_bias: bool, proj_bias: bool):
    nc = bass.Bass("TRN2", target_bir_lowering=False, debug=False, num_devices=N_CORES)

    xq8 = nc.dram_tensor("xq8", [3 * P, 2 * S], F8, kind="ExternalInput").ap()
    xkb = nc.dram_tensor("xkb", [D, S], BF16, kind="ExternalInput").ap()
    wq8 = nc.dram_tensor("wq8", [3 * P, 2 * 2 * DL], F8, kind="ExternalInput").ap()
    wvb = nc.dram_tensor("wvb", [D, DL], BF16, kind="ExternalInput").ap()
    wpb = nc.dram_tensor("wpb", [DL, D], BF16, kind="ExternalInput").ap()
    ba8 = nc.dram_tensor("ba8", [P, 8], F32, kind="ExternalInput").ap()
    bpb = nc.dram_tensor("bpb", [P, D], F32, kind="ExternalInput").ap()
    mtri = nc.dram_tensor("mtri", [P, P], F32, kind="ExternalInput").ap()
    idn = nc.dram_tensor("idn", [P, P], BF16, kind="ExternalInput").ap()
    out = nc.dram_tensor("out", [S, D], F32, kind="ExternalOutput").ap()
    ar_in = nc.dram_tensor("ar_in", [S, D], F32).ap()
    ar_outs = [nc.dram_tensor(f"ar_out{c}", [S // 2, D], F32).ap() for c in range(2)]

    with tile.TileContext(nc) as tc:
        with (
            tc.tile_pool(name="wgt", bufs=1) as wpool,
            tc.tile_pool(name="qk", bufs=1) as qkpool,
            tc.tile_pool(name="vpr", bufs=1) as vpool,
            tc.tile_pool(name="pt", bufs=6) as ptpool,
            tc.tile_pool(name="yc", bufs=2) as ycpool,
            tc.tile_pool(name="ytp", bufs=2) as ytpool,
            tc.tile_pool(name="nrm", bufs=2) as nrmpool,
            tc.tile_pool(name="ob", bufs=3) as obpool,
            tc.tile_pool(name="scp", bufs=2, space="PSUM") as scpool,
            tc.tile_pool(name="avp", bufs=1, space="PSUM") as avpool,
            tc.tile_pool(name="trp", bufs=1, space="PSUM") as trpool,
            tc.tile_pool(name="prj", bufs=1, space="PSUM") as prpool,
        ):
            # ---- phase 0: load weights/constants ----
            xq = []   # fp8 x d-pair tiles [128, 2, S]
            for t in range(NDP):
                xt = wpool.tile([P, 2 * S], F8, tag=f"xq{t}")
                nc.sync.dma_start(xt[:, 0:S], xq8[bass.ts(t, P), 0:S])
                nc.sync.dma_start(xt[:, S:], xq8[bass.ts(t, P), S:])
                xq.append(xt)
            wq = []   # fp8 wa-qk d-pair tiles [128, 2, 768]
            for t in range(NDP):
                wt = wpool.tile([P, 2 * 2 * DL], F8, tag=f"wq{t}")
                nc.sync.dma_start(wt[:], wq8[bass.ts(t, P), :])
                wq.append(wt)
            xk = []   # bf16 x^T tiles [128, S] (V stationary)
            for t in range(NDT):
                xt = wpool.tile([P, S], BF16, tag=f"xk{t}")
                nc.sync.dma_start(xt[:, 0 : S // 2], xkb[bass.ts(t, P), 0 : S // 2])
                xk.append(xt)
            for t in range(NDT):
                nc.sync.dma_start(xk[t][:, S // 2 :], xkb[bass.ts(t, P), S // 2 :])
            wv = []
            for t in range(NDT):
                wt = wpool.tile([P, DL], BF16, tag=f"wv{t}")
                nc.sync.dma_start(wt[:], wvb[bass.ts(t, P), :])
                wv.append(wt)
            wpp = []
            for p in range(NP):
                wt = wpool.tile([P, D], BF16, tag=f"wp{p}")
                nc.sync.dma_start(wt[:], wpb[bass.ts(p, P), :])
                wpp.append(wt)
            ba_sb = wpool.tile([P, 8], F32, tag="ba")
            if qk_bias:
                nc.sync.dma_start(ba_sb[:], ba8[:])
            bp_sb = wpool.tile([P, D], F32, tag="bp")
            if proj_bias:
                nc.sync.dma_start(bp_sb[:], bpb[:])
            mt_sb = wpool.tile([P, P], F32, tag="mtri")
            nc.sync.dma_start(mt_sb[:], mtri[:])
            id_sb = wpool.tile([P, P], BF16, tag="idn")
            nc.sync.dma_start(id_sb[:], idn[:])
            # prewarm ScalarE's exp table
            warm_sb = wpool.tile([1, 2], F32, tag="warm")
            nc.vector.memset(warm_sb[:], 0.0)
            nc.scalar.activation(warm_sb[:, 1:2], warm_sb[:, 0:1],
                                 mybir.ActivationFunctionType.Exp)

            # ---- phase 1: Q,K -> fp8 [32,2,S]-layout tiles ----
            # qraw m-tiles 0..2 = Q pairs, 3..5 = K pairs, [128, S] fp8
            # (partitions: head A dims 0:64, head B dims 64:128)
            qt8 = [qkpool.tile([64, 2 * S], F8, tag=f"qt8{p}", name=f"qt8{p}") for p in range(NP)]
            kt8 = [qkpool.tile([64, 2 * S], F8, tag=f"kt8{p}", name=f"kt8{p}") for p in range(NP)]

            def emit_qk(m):
                qraw = qkpool.tile([P, S], F8, tag=f"qraw{m}")
                for n in range(S // QT):
                    ps = scpool.tile([P, 1024], F32, tag="sc")
                    for t in range(NDP):
                        nc.tensor.matmul(
                            ps[:, 0:QT],
                            lhsT=wq[t][:].rearrange("p (u m) -> p u m", u=2)[
                                :, :, bass.ts(m, P)],
                            rhs=xq[t][:].rearrange("p (u s) -> p u s", u=2)[
                                :, :, bass.ts(n, QT)],
                            start=(t == 0), stop=(t == NDP - 1), perf_mode=DR,
                        )
                    if qk_bias:
                        nc.vector.tensor_scalar_add(
                            qraw[:, bass.ts(n, QT)], ps[:, 0:QT], ba_sb[:, m : m + 1])
                    else:
                        nc.vector.tensor_copy(qraw[:, bass.ts(n, QT)], ps[:, 0:QT])
                # relayout [128, S] -> [32, 2, S] per head (A rows 0:32, B 32:64)
                dst = qt8[m] if m < NP else kt8[m - NP]
                for h in range(2):
                    for u in range(2):
                        nc.sync.dma_start(
                            dst[32 * h : 32 * h + 32, u * S : (u + 1) * S],
                            qraw[64 * h + 32 * u : 64 * h + 32 * u + 32, :],
                        )

            # V s-tiles: [128, 780] fp8 = [hi 6x65 | lo 6x65], col 64-of-65:
            # hi=1 (softmax denominator via ones trick), lo=0
            v_t = [None] * NS

            def emit_v(s):
                ps = prpool.tile([P, 512], F32, tag="prj")
                for t in range(NDT):
                    nc.tensor.matmul(
                        ps[:, 0:DL],
                        lhsT=xk[t][:, bass.ts(s, P)],
                        rhs=wv[t][:],
                        start=(t == 0), stop=(t == NDT - 1),
                    )
                vt = vpool.tile([P, 2 * HL * 65], F8, tag=f"v{s}")
                vt4 = vt[:].rearrange("p (u h x) -> p u h x", u=2, h=HL)
                ps3 = ps[:, 0:DL].rearrange("p (h x) -> p h x", h=HL)
                nc.vector.tensor_copy(vt4[:, 0, :, 0:HD], ps3)
                nc.vector.tensor_sub(vt4[:, 1, :, 0:HD], ps3, vt4[:, 0, :, 0:HD])
                nc.vector.memset(vt4[:, 0, :, HD : HD + 1], 1.0)
                nc.vector.memset(vt4[:, 1, :, HD : HD + 1], 0.0)
                v_t[s] = vt

            ytp_t = [None] * NP

            def q_ap(p, h, q0, n):
                # rhs AP [32, 2, n] at q offset q0 for head h of pair p
                return qt8[p][32 * h : 32 * h + 32, :].rearrange(
                    "p (u s) -> p u s", u=2)[:, :, q0 : q0 + n]

            def k_ap(p, h, j):
                return kt8[p][32 * h : 32 * h + 32, :].rearrange(
                    "p (u s) -> p u s", u=2)[:, :, bass.ts(j, P)]

            def emit_attn(i, p):
                av = avpool.tile([P, 1024], F32, tag="av")
                av_started = [False, False]

                def av_mm(h, cc, j, pt_ap, stop):
                    # start=True zero-fills the whole PSUM bank, so exactly
                    # the first matmul touching each head's bank carries it
                    hh = 2 * p + h
                    nc.tensor.matmul(
                        av[:, 512 * h + cc * 65 : 512 * h + cc * 65 + 65],
                        lhsT=pt_ap.unsqueeze(1).broadcast_to((P, 2, P)),
                        rhs=v_t[j][:].rearrange("p (u n) -> p u n", u=2)[
                            :, :, 65 * hh : 65 * hh + 65],
                        start=not av_started[h], stop=stop, perf_mode=DR,
                        skip_group_check=True,
                    )
                    av_started[h] = True

                # off-diagonal full groups (pairs of j blocks)
                for g in range(2 * i):
                    j0, j1 = 2 * g, 2 * g + 1
                    pts = []
                    for h in range(2):
                        sc = scpool.tile([P, 1024], F32, tag="sc")
                        for jj, off in ((j0, 0), (j1, QT)):
                            nc.tensor.matmul(
                                sc[:, off : off + QT],
                                lhsT=k_ap(p, h, jj),
                                rhs=q_ap(p, h, i * QT, QT),
                                start=True, stop=True, perf_mode=DR,
                            )
                        pt = ptpool.tile([P, 1024], F8, tag="pt")
                        nc.scalar.activation(pt[:], sc[:],
                                             mybir.ActivationFunctionType.Exp,
                                             scale=ESC)
                        pts.append(pt)
                    for h in range(2):
                        for cc in range(4):
                            for jj, off in ((j0, 0), (j1, QT)):
                                av_mm(h, cc, jj,
                                      pts[h][:, off + cc * P : off + cc * P + P],
                                      stop=False)

                # diagonal: 8 staircase chunks (A 0:512 | B 512:1024) + mask
                scd = scpool.tile([P, 1024], F32, tag="sc")
                for h in range(2):
                    for mi in range(4):
                        nc.tensor.matmul(
                            scd[:, h * QT + mi * P : h * QT + mi * P + P],
                            lhsT=k_ap(p, h, 4 * i + mi),
                            rhs=q_ap(p, h, i * QT + mi * P, P),
                            start=(mi == 0), stop=True, perf_mode=DR,
                            skip_group_check=True,
                        )
                scd8 = scd[:].rearrange("p (c n) -> p c n", c=8)
                nc.vector.tensor_add(
                    scd8, scd8, mt_sb[:].unsqueeze(1).broadcast_to((P, 8, P)))
                ptD = ptpool.tile([P, 1024], F8, tag="pt")
                nc.scalar.activation(ptD[:], scd[:],
                                     mybir.ActivationFunctionType.Exp, scale=ESC)
                # diagonal remainders (mask-free): mi0@0 w384, mi2@384 w128,
                # mi1@512 w256  (bank-crossing-free packing)
                ptR = []
                for h in range(2):
                    scr = scpool.tile([P, 1024], F32, tag="sc")
                    for mi in range(3):
                        w = REM_W[mi]
                        off = REM_OFF[mi]
                        nc.tensor.matmul(
                            scr[:, off : off + w],
                            lhsT=k_ap(p, h, 4 * i + mi),
                            rhs=q_ap(p, h, i * QT + mi * P + P, w),
                            start=(mi < 2), stop=True, perf_mode=DR,
                            skip_group_check=True,
                        )
                    ptr = ptpool.tile([P, 1024], F8, tag="pt")
                    nc.scalar.activation(ptr[:, 0:768], scr[:, 0:768],
                                         mybir.ActivationFunctionType.Exp,
                                         scale=ESC)
                    ptR.append(ptr)
                for h in range(2):
                    for cc in range(4):
                        for mi in range(cc):
                            av_mm(h, cc, 4 * i + mi,
                                  ptR[h][:, REM_OFF[mi] + (cc - mi - 1) * P :
                                         REM_OFF[mi] + (cc - mi) * P],
                                  stop=False)
                        av_mm(h, cc, 4 * i + cc,
                              ptD[:, h * QT + cc * P : h * QT + cc * P + P],
                              stop=True)

                # normalize (per-partition scalar) + build y chunks
                rc = nrmpool.tile([P, 8], F32, tag="rc")
                yc = ycpool.tile([P, QT], BF16, tag="yc")
                yc4 = yc[:].rearrange("p (c n) -> p c n", c=4)
                for h in range(2):
                    av4 = av[:, 512 * h : 512 * h + 260].rearrange(
                        "p (c n) -> p c n", c=4)
                    nc.vector.reciprocal(
                        rc[:, 4 * h : 4 * h + 4].unsqueeze(2), av4[:, :, 64:65])
                    nc.vector.tensor_mul(
                        yc4[:, :, 64 * h : 64 * h + 64],
                        av4[:, :, 0:64],
                        rc[:, 4 * h : 4 * h + 4].unsqueeze(2).broadcast_to((P, 4, 64)),
                    )
                ytp = ytpool.tile([P, QT], BF16, tag=f"ytp{p}")
                tp = trpool.tile([P, 1024], BF16, tag="tr")
                for cc in range(4):
                    nc.tensor.matmul(
                        tp[:, cc * P : cc * P + P], yc[:, cc * P : cc * P + P],
                        id_sb[:], is_transpose=True,
                        start=(cc == 0), stop=True, skip_group_check=True)
                nc.vector.tensor_copy(ytp[:], tp[:, 0:QT])
                ytp_t[p] = ytp

            def emit_copy(c, r):
                rr = r - c * (S // 2)
                oc = obpool.tile([P, D], F32, tag="oc")
                nc.sync.dma_start(oc[:], ar_outs[c][rr : rr + P, :])
                nc.sync.dma_start(out[r : r + P, :], oc[:])

            def emit_ar(c):
                r0 = c * (S // 2)
                nc.gpsimd.collective_compute(
                    "AllReduce",
                    mybir.AluOpType.add,
                    replica_groups=[[0, 1], [2, 3], [4, 5], [6, 7]],
                    ins=[ar_in[r0 : r0 + S // 2, :].opt()],
                    outs=[ar_outs[c][:].opt()],
                )

            def emit_proj(i):
                for ss in range(4):
                    row = i * QT + ss * P
                    if proj_bias:
                        ps = prpool.tile([P, D], F32, tag="prjw")
                        for half in range(2):
                            for p in range(NP):
                                nc.tensor.matmul(
                                    ps[:, half * DL : half * DL + DL],
                                    lhsT=ytp_t[p][:, ss * P : ss * P + P],
                                    rhs=wpp[p][:, half * DL : half * DL + DL],
                                    start=(p == 0), stop=(p == NP - 1),
                                )
                        ob = obpool.tile([P, D], F32, tag="ob")
                        nc.vector.tensor_add(ob[:], ps[:], bp_sb[:])
                        nc.sync.dma_start(ar_in[row : row + P, :], ob[:])
                    else:
                        ob = obpool.tile([P, D], F32, tag="ob")
                        for half in range(2):
                            ps = prpool.tile([P, 512], F32, tag="prj")
                            for p in range(NP):
                                nc.tensor.matmul(
                                    ps[:, 0:DL],
                                    lhsT=ytp_t[p][:, ss * P : ss * P + P],
                                    rhs=wpp[p][:, half * DL : half * DL + DL],
                                    start=(p == 0), stop=(p == NP - 1),
                                )
                            nc.vector.tensor_copy(
                                ob[:, half * DL : half * DL + DL], ps[:, 0:DL])
                        nc.sync.dma_start(ar_in[row : row + P, :], ob[:])

            # ---- main schedule ----
            for m in range(2 * NP):
                emit_qk(m)
            for s in range(4):
                emit_v(s)
            for i in range(NI):
                for p in range(NP):
                    emit_attn(i, p)
                if i + 1 < NI:
                    for s in range(4 * (i + 1), 4 * (i + 2)):
                        emit_v(s)
                emit_proj(i)
                if i == 1:
                    emit_ar(0)
                if i == 3:
                    for r in range(0, S // 2, P):
                        emit_copy(0, r)
                    emit_ar(1)
                    for r in range(S // 2, S, P):
                        emit_copy(1, r)

    _legalize_waits(nc)
    return nc


_NC_CACHE = {}


def _get_nc(qk_bias=False, proj_bias=False):
    key = (qk_bias, proj_bias)
    if key not in _NC_CACHE:
        _NC_CACHE[key] = _build(qk_bias, proj_bias)
    return _NC_CACHE[key]


def _prep_inputs(x, W_attn, b_attn, W_proj, b_proj):
    bf = ml_dtypes.bfloat16
    f8 = ml_dtypes.float8_e4m3
    x = np.asarray(x, np.float32)
    W_attn = np.asarray(W_attn, np.float32)
    b_attn = np.asarray(b_attn, np.float32)
    W_proj = np.asarray(W_proj, np.float32)
    b_proj = np.asarray(b_proj, np.float32)

    k_idx = np.arange(P)[:, None]
    q_idx = np.arange(P)[None, :]
    mtri = np.where(q_idx >= k_idx, 0.0, MASKC).astype(np.float32)
    idn = np.eye(P).astype(bf)

    Wq = W_attn[:, 0:D]
    Wk = W_attn[:, D : 2 * D]
    Wv = W_attn[:, 2 * D :]

    in_maps = []
    meta = {}
    for c in range(N_CORES):
        b, g = divmod(c, 2)
        cols = slice(DL * g, DL * g + DL)
        xb = x[b]                                   # [S, D]
        # fp8 x in d-pair layout [3*128, 2*S]
        xq8 = np.empty((3 * P, 2 * S), f8)
        for t in range(NDP):
            for u in range(2):
                xq8[t * P : (t + 1) * P, u * S : (u + 1) * S] = (
                    xb[:, 256 * t + P * u : 256 * t + P * u + P].T.astype(f8))
        xkb = np.ascontiguousarray(xb.T).astype(bf)
        # fp8 W_attn q,k (scaled) in d-pair layout [3*128, 2*768]
        wa_qk = np.concatenate([Wq[:, cols], Wk[:, cols]], axis=1) * SA  # [D, 768]
        wq8 = np.empty((3 * P, 2 * 2 * DL), f8)
        for t in range(NDP):
            for u in range(2):
                wq8[t * P : (t + 1) * P, u * 2 * DL : (u + 1) * 2 * DL] = (
                    wa_qk[256 * t + P * u : 256 * t + P * u + P, :].astype(f8))
        wvb = np.ascontiguousarray(Wv[:, cols]).astype(bf)
        wpb = np.ascontiguousarray(W_proj[cols, :]).astype(bf)
        # qk bias (scaled): ba8[p, m] = SA * b[col m*128+p]
        ba_qk = np.concatenate([b_attn[0:D][cols], b_attn[D : 2 * D][cols]]) * SA
        ba8 = np.zeros((P, 8), np.float32)
        ba8[:, :6] = ba_qk.reshape(6, P).T
        # b_proj (+ v-bias folded) added once per row: only group 0 carries it
        bv = b_attn[2 * D :][cols]
        bp_eff = bv @ W_proj[cols, :] + (b_proj if g == 0 else 0.0)
        bpb = np.ascontiguousarray(
            np.broadcast_to(bp_eff.astype(np.float32), (P, D)))
        in_maps.append({
            "xq8": xq8, "xkb": xkb, "wq8": wq8, "wvb": wvb, "wpb": wpb,
            "ba8": ba8, "bpb": bpb, "mtri": mtri, "idn": idn,
        })
        meta.setdefault("qk_bias", bool(np.any(ba_qk != 0.0)))
        meta["proj_bias"] = meta.get("proj_bias", False) or bool(
            np.any(bp_eff != 0.0))
    return in_maps, meta


def kernel(x, W_attn, b_attn, W_proj, b_proj):
    in_maps, meta = _prep_inputs(x, W_attn, b_attn, W_proj, b_proj)
    nc = _get_nc(meta["qk_bias"], meta["proj_bias"])
    res = run_bass_kernel_spmd(nc, in_maps, list(range(N_CORES)))
    out = np.stack([res.results[2 * b]["out"] for b in range(B)])
    return out.astype(np.float32)
